# revision 19
# baseline (speedup 1.0000x reference)
"""Trainium2 Bass kernel for nn_DiffusionActionHead (MoE-style category routing).

Strategy (host side, inside kernel()):
  - The network splits into a per-TOKEN bulk path and two per-ITEM vector
    paths. The per-item paths (state encoder: 1 token/item; the timestep
    sinusoid's contribution tau @ ae_W2[EMB:]: identical for all T tokens of
    an item) are computed exactly on host in fp32/64 — keeping them on device
    would cost ~7.6MB/category of HBM weight traffic to produce two
    1536-vectors per item. The action-encoder first layer is folded into the
    second (host, per category): F = ae_W1 @ ae_W2[:EMB] (rank-32
    bottleneck), so the device computes, per token:
        z = actions @ F + tt[b];  x2 = silu(z);  out = x2 @ W3
    where tt[b] = tau[b] @ ae_W2[EMB:] + ae_b1 @ ae_W2[:EMB] + ae_b2.
  - W3 (the dominant remaining traffic, 4.7MB/category bf16) is quantized to
    fp8 e3m4 with a per-category power-of-2 scale s_g chosen so
    max|W3·s_g| <= 15.5; the device computes x2 @ (W3·s_g) with a mixed
    bf16 x fp8 matmul (PSUM fp32) and the host divides by s_g during
    unsharding (exact). Everything else ships bf16. Measured pipeline rel
    err ~1.37e-2 (gate 2e-2), stable across seeds.
  - Routing: group the B items by cat_id into chunks of <=4 items (128
    tokens); each chunk splits into 3 output-column thirds (512 cols of W3,
    786KB fp8) = uniform "units". Units are sorted by item count (desc) and
    dealt round-robin over the 8 cores, so slot-row r holds units of similar
    token count; the program bakes a per-slot token capacity cap[r] (the row
    max), and matmuls move only cap tokens — PE work scales with real
    tokens while the weight DMA (the roofline) is unaffected.
  - tt is injected into z via 4 indicator rows appended to actionsT
    (tokens of item i select tt_i), so the whole z phase is ONE matmul per
    128-feature tile: [36, 128]^T @ [36, cap].
  - DMA transfers carry a large fixed cost, so they are aggressively
    batched: w3 ships in 4-slot groups split across the SP and ACT HWDGE
    rings (2 transfers per ring per 8 slots), pin (F_aug+actionsT+tt) in
    8-slot groups (1 transfer), outputs in 4-slot groups on the DVE ring.

Device program per slot (raw Bass, manual semaphores):
  Z    12x mm: pZ bank[t%3] col128*(t//3) = F_aug_chunk^T @ actsT_aug
       (emitted bank-major; PSUM [128 feat, cap]), then 3x fused Silu,
       one per bank, into the bf16 x2T staging (bank-major chunk order)
  AE3  4oc x 12k: pO[:, oc*cap:+cap] += W3q[k,oc]^T @ x2T[k]  (fp8 x bf16)
  OUT  DVE copy pO -> s_out group staging; one DMA per 4 slots
"""
import sys

sys.path.insert(0, "/opt/trn_rl_repo")

import contextlib
import numpy as np
import ml_dtypes

import concourse.bass as bass
import concourse.mybir as mybir
from concourse.bass_utils import run_bass_kernel_spmd

F32 = mybir.dt.float32
BF16 = mybir.dt.bfloat16
FP8 = mybir.dt.float8e3
NPBF16 = ml_dtypes.bfloat16
NPFP8 = ml_dtypes.float8_e3m4
AF = mybir.ActivationFunctionType

E, STATE_DIM, ACT_DIM, HID, EMB = 32, 64, 32, 1024, 1536
B, T = 32, 32
N_CORES = 8
ITEMS_PER_SLOT = 4      # token capacity tile = 4*32 = 128 tokens
NTHIRD = 3              # W3 output-column split -> unit granularity
OCW = EMB // NTHIRD     # 512 outcols per unit
KCH = EMB // 128        # 12 feature chunks
HCOL = KCH * OCW // 2   # 3072: per-slot w3 half (k-chunks 0:6 / 6:12)
GW = 4                  # w3 slots per DMA group
GP = 8                  # pin slots per DMA group
GO = 4                  # out slots per DMA group
RS = 2                  # w3 ring depth in groups (= GW*RS slots of runway)
PIN_F = 0               # pin cols 0:1536   = F rows + tt rows (F_aug)
PIN_A = EMB             # pin cols 1536:1664 = actionsT + indicator rows
PIN_W = EMB + 128
FP8MAX = 15.5           # float8_e3m4 max normal
# x2T column order is bank-major: feature chunk c sits at column slot
# (c % 3) * 4 + (c // 3), so one fused Silu per PSUM bank writes 4
# contiguous chunk slots.
XPOS = [(c % 3) * 4 + (c // 3) for c in range(KCH)]


def _sinusoid(ts):
    half = EMB // 2
    div = np.exp(-np.log(np.float64(10000.0)) * np.arange(half) / np.float64(half))
    ang = ts.astype(np.float64)[:, None] * div[None, :]
    return np.concatenate([np.sin(ang), np.cos(ang)], axis=1)


# ---------------------------------------------------------------------------
# Build-time plan. Ops live in engine streams: "dma" (SP: w3 half A),
# "actq" (ACT: w3 half B DMA + Silu), "dve" (DVE: pin DMA, psum->sbuf
# copies, out DMA), "pe" (matmuls). Every DMA incs its own per-buffer sem by
# 16; every PE op incs s_pe by 1; ACT s_act; DVE s_dve. Cross-engine deps
# become wait_ge ops computed from per-buffer writer/reader tags.
# ---------------------------------------------------------------------------
class _Buf:
    __slots__ = ("writer", "readers")

    def __init__(self):
        self.writer = None
        self.readers = []


class _Plan:
    def __init__(self):
        self.dma = []
        self.actq = []
        self.dve = []
        self.gq = []
        self.pe = []
        self.counts = {}

    def emit(self, stream, sem, mult, op, in_bufs, out_buf, force_wait=False):
        self.counts[sem] = self.counts.get(sem, 0) + 1
        tag = (sem, self.counts[sem] * mult, stream)
        deps = []
        for b in in_bufs:
            if b.writer is not None:
                deps.append(b.writer)
        if out_buf is not None:
            deps.extend(out_buf.readers)
            if out_buf.writer is not None:
                deps.append(out_buf.writer)
        m = {}
        for dsem, dval, dstream in deps:
            if dstream == stream and not force_wait:
                continue
            m[dsem] = max(m.get(dsem, 0), dval)
        op["waits"] = m
        getattr(self, stream).append(op)
        for b in in_bufs:
            b.readers.append(tag)
        if out_buf is not None:
            out_buf.writer = tag
            out_buf.readers = []


def build(caps, reps=1, probe=None):
    nslot = len(caps)
    ngw = -(-nslot // GW)
    ngp = -(-nslot // GP)
    ngo = -(-nslot // GO)
    nc = bass.Bass()
    P = nc.declare_dram_parameter

    w3a = P("w3a", [ngw, 128, GW * HCOL], FP8, isOutput=False)
    w3b = P("w3b", [ngw, 128, GW * HCOL], FP8, isOutput=False)
    pin = P("pin", [ngp, 36, GP * PIN_W], BF16, isOutput=False)
    ao = P("ao", [ngo, 128, GO * OCW], BF16, isOutput=True)

    with contextlib.ExitStack() as es:
        ec = es.enter_context
        ring_a = [ec(nc.sbuf_tensor(f"rga{i}", [128, GW * HCOL], FP8)) for i in range(RS)]
        ring_b = [ec(nc.sbuf_tensor(f"rgb{i}", [128, GW * HCOL], FP8)) for i in range(RS)]
        pin_b = [ec(nc.sbuf_tensor(f"pin{i}", [36, GP * PIN_W], BF16)) for i in range(2)]
        s_x2T = [ec(nc.sbuf_tensor(f"x2T{i}", [128, EMB], BF16)) for i in range(2)]
        s_out = [ec(nc.sbuf_tensor(f"sout{i}", [128, GO * OCW], BF16)) for i in range(2)]
        pZ = [ec(nc.psum_tensor(f"pZ{i}", [128, 512], F32)) for i in range(3)]
        pO = [ec(nc.psum_tensor(f"pO{i}", [128, 512], F32)) for i in range(2)]
        s_pe = ec(nc.semaphore("s_pe"))
        s_act = ec(nc.semaphore("s_act"))
        s_dve = ec(nc.semaphore("s_dve"))
        block = ec(nc.Block())

        # ---------------- plan ----------------
        pl = _Plan()
        bufs = {
            "rga": [_Buf() for _ in range(RS)],
            "rgb": [_Buf() for _ in range(RS)],
            "pin": [_Buf() for _ in range(2)],
            "x2T": [[_Buf() for _ in range(3)] for _ in range(2)],
            "out": [_Buf() for _ in range(2)],
            # PSUM tracked at bank granularity: concurrent PE write + ACT/DVE
            # read of one bank is fatal (P10).
            "pZ": [_Buf() for _ in range(3)],
            "pO": [_Buf() for _ in range(2)],
        }

        def dma(stream, pfx, dst, dst_sl, src, src_sl, in_bufs, out_buf, key):
            pl.emit(stream, pfx + key, 16,
                    {"kind": "dma", "dst": dst, "dst_sl": dst_sl, "src": src,
                     "src_sl": src_sl, "key": pfx + key}, in_bufs, out_buf)

        def mm(out, out_sl, lhs, lhs_sl, rhs, rhs_sl, start, stop, in_bufs, out_buf):
            pl.emit("pe", "pe", 1,
                    {"kind": "mm", "out": out, "out_sl": out_sl, "lhs": lhs,
                     "lhs_sl": lhs_sl, "rhs": rhs, "rhs_sl": rhs_sl,
                     "start": start, "stop": stop}, in_bufs, out_buf)

        def act(out, out_sl, in_, in_sl, func, in_bufs, out_buf):
            pl.emit("actq", "act", 1,
                    {"kind": "act", "out": out, "out_sl": out_sl, "in": in_,
                     "in_sl": in_sl, "func": func}, in_bufs, out_buf)

        def dve(out, out_sl, in_, in_sl, in_bufs, out_buf):
            pl.emit("dve", "dve", 1,
                    {"kind": "copy", "out": out, "out_sl": out_sl, "in": in_,
                     "in_sl": in_sl}, in_bufs, out_buf)

        def emit_slot(gs, s):
            cap = caps[s]
            sb = gs % 2
            gw, wpos = s // GW, s % GW
            gp, ppos = s // GP, s % GP
            go, opos = s // GO, s % GO
            rep = gs // nslot
            g_gw = rep * ngw + gw            # global w3 group index
            rg = g_gw % RS
            pb = (rep * ngp + gp) % 2        # pin buffer parity by global group
            ob = (rep * ngo + go) % 2        # out staging parity by global group

            if wpos == 0:
                wcols = min(GW, nslot - gw * GW) * HCOL
                dma("dma", "dma:", "ring_a", (rg, np.s_[:, 0:wcols]),
                    "w3a", np.s_[gw, :, 0:wcols], [], bufs["rga"][rg], f"w3a{rg}")
                dma("actq", "dmo:", "ring_b", (rg, np.s_[:, 0:wcols]),
                    "w3b", np.s_[gw, :, 0:wcols], [], bufs["rgb"][rg], f"w3b{rg}")
            if ppos == 0:
                pcols = min(GP, nslot - gp * GP) * PIN_W
                dma("dma", "dma:", "pin_b", (pb, np.s_[:, 0:pcols]),
                    "pin", np.s_[gp, :, 0:pcols], [], bufs["pin"][pb], f"pin{pb}")

            po = ppos * PIN_W
            # ---- Z, bank-major: 4 feature chunks -> one bank -> one Silu.
            # Chunk c sits at pZ bank c%3, col 128*(c//3); its silu result
            # lands at x2T column slot XPOS[c]. ----
            for bk in range(3):
                for q in range(4):
                    c = bk + 3 * q
                    mm("pZ", (bk, np.s_[:, q * 128:q * 128 + cap]),
                       "pin_b", (pb, np.s_[:, po + PIN_F + c * 128:po + PIN_F + (c + 1) * 128]),
                       "pin_b", (pb, np.s_[:, po + PIN_A:po + PIN_A + cap]),
                       True, True, [bufs["pin"][pb]], bufs["pZ"][bk])
                act("s_x2T", (sb, np.s_[:, bk * 512:(bk + 1) * 512]),
                    "pZ", (bk, np.s_[:, 0:512]),
                    AF.Silu, [bufs["pZ"][bk]], bufs["x2T"][sb][bk])

            # ---- AE3: W3 chunks stationary, tokens moving (cost ~ cap).
            # oc outer so each PSUM accumulation group (12 k-steps) completes
            # before the next group in the same bank starts. ----
            wbase = wpos * HCOL
            for oc in range(4):
                for k in range(KCH):
                    rn, rb = (("ring_a", bufs["rga"][rg]) if k < 6
                              else ("ring_b", bufs["rgb"][rg]))
                    kq = k if k < 6 else k - 6
                    mm("pO", (sb, np.s_[:, oc * cap:(oc + 1) * cap]),
                       rn, (rg, np.s_[:, wbase + kq * OCW + oc * 128:wbase + kq * OCW + (oc + 1) * 128]),
                       "s_x2T", (sb, np.s_[:, XPOS[k] * 128:XPOS[k] * 128 + cap]),
                       k == 0, k == KCH - 1,
                       [rb, bufs["x2T"][sb][k % 3]], bufs["pO"][sb])

            dve("s_out", (ob, np.s_[:, opos * OCW:opos * OCW + 4 * cap]),
                "pO", (sb, np.s_[:, 0:4 * cap]),
                [bufs["pO"][sb]], bufs["out"][ob])
            if opos == GO - 1 or s == nslot - 1:
                dma("gq", "dmg:", "ao", np.s_[go, :, :],
                    "s_out", (ob, np.s_[:, :]), [bufs["out"][ob]], None,
                    f"out{ob}")

        for rep in range(reps):
            for s in range(nslot):
                emit_slot(rep * nslot + s, s)

        # ---------------- emit ----------------
        if probe == "pe":
            pl.dma, pl.actq, pl.dve, pl.gq = [], [], [], []
            for o in pl.pe:
                o["waits"] = {}
        if probe == "act":
            pl.dma, pl.pe, pl.dve, pl.gq = [], [], [], []
            pl.actq = [o for o in pl.actq if o["kind"] == "act"]
            for o in pl.actq:
                o["waits"] = {}
        if probe == "dma":
            pl.pe, pl.dve = [], []
            pl.actq = [o for o in pl.actq if o["kind"] == "dma"]
            pl.gq = [o for o in pl.gq if o.get("dst") != "ao"]
            kc = {}
            for lst in (pl.dma, pl.actq, pl.gq):
                for o in lst:
                    k = o["key"]
                    o["waits"] = {k: 16 * kc[k]} if kc.get(k, 0) > 0 else {}
                    kc[k] = kc.get(k, 0) + 1

        dma_sems = {k: ec(nc.semaphore("sem_" + k.replace(":", "_")))
                    for k in pl.counts if k.startswith(("dma:", "dmo:", "dmg:"))}

        tensors = {"ring_a": ring_a, "ring_b": ring_b, "pin_b": pin_b,
                   "s_x2T": s_x2T, "s_out": s_out, "pZ": pZ, "pO": pO,
                   "w3a": w3a, "w3b": w3b, "pin": pin, "ao": ao}

        def ap(name, sl):
            t = tensors[name]
            if isinstance(t, list):
                i, s2 = sl
                return t[i][s2]
            return t[sl]

        sems = {"pe": s_pe, "act": s_act, "dve": s_dve}

        def make_waiter(eng):
            hw = {}

            def wait(wmap):
                for sname in sorted(wmap):
                    val = wmap[sname]
                    if hw.get(sname, 0) >= val:
                        continue
                    hw[sname] = val
                    h = sems[sname] if sname in sems else dma_sems[sname]
                    eng.wait_ge(h, val)

            return wait

        def run_stream(eng, ops):
            wait = make_waiter(eng)
            cnt = {}
            for op in ops:
                wait(op["waits"])
                if op["kind"] == "dma":
                    k = op["key"]
                    cnt[k] = cnt.get(k, 0) + 16
                    eng.dma_start(out=ap(op["dst"], op["dst_sl"]),
                                  in_=ap(op["src"], op["src_sl"])).then_inc(dma_sems[k], 16)
                elif op["kind"] == "mm":
                    eng.matmul(ap(op["out"], op["out_sl"]), ap(op["lhs"], op["lhs_sl"]),
                               ap(op["rhs"], op["rhs_sl"]), start=op["start"],
                               stop=op["stop"]).then_inc(s_pe, 1)
                elif op["kind"] == "act":
                    eng.activation(ap(op["out"], op["out_sl"]), ap(op["in"], op["in_sl"]),
                                   op["func"]).then_inc(s_act, 1)
                else:
                    eng.tensor_copy(ap(op["out"], op["out_sl"]),
                                    ap(op["in"], op["in_sl"])).then_inc(s_dve, 1)
            for k, v in sorted(cnt.items()):
                eng.wait_ge(dma_sems[k], v)

        @block.sync
        def _(sync):
            run_stream(sync, pl.dma)

        @block.tensor
        def _(pe):
            run_stream(pe, pl.pe)

        @block.scalar
        def _(a):
            run_stream(a, pl.actq)

        @block.vector
        def _(v):
            run_stream(v, pl.dve)

        @block.gpsimd
        def _(g):
            run_stream(g, pl.gq)

    return nc


# ---------------------------------------------------------------------------
# Host-side routing, preprocessing, execution, unsharding
# ---------------------------------------------------------------------------
def plan_units(cat_ids):
    """Units (cat, items<=4, third), sorted by item count desc for cap rows."""
    order = {}
    for b, g in enumerate(cat_ids.tolist()):
        order.setdefault(g, []).append(b)
    chunks = []
    for g in sorted(order):
        items = order[g]
        for i0 in range(0, len(items), ITEMS_PER_SLOT):
            chunks.append((g, items[i0:i0 + ITEMS_PER_SLOT]))
    chunks.sort(key=lambda c: -len(c[1]))
    units = [(g, items, h) for (g, items) in chunks for h in range(NTHIRD)]
    return units


def route(cat_ids):
    units = plan_units(cat_ids)
    nslot = max(1, -(-len(units) // N_CORES))
    per_core = [[None] * nslot for _ in range(N_CORES)]
    for i, u in enumerate(units):
        per_core[i % N_CORES][i // N_CORES] = u
    caps = [T * len(units[min(s * N_CORES, len(units) - 1)][1]) for s in range(nslot)]
    return units, per_core, caps


def make_inputs(units_c, caps, actions_bf, pre):
    nslot = len(caps)
    ngw = -(-nslot // GW)
    ngp = -(-nslot // GP)
    w3a = np.zeros((ngw, 128, GW * HCOL), NPFP8)
    w3b = np.zeros((ngw, 128, GW * HCOL), NPFP8)
    pin = np.zeros((ngp, 36, GP * PIN_W), NPBF16)
    for s, u in enumerate(units_c):
        if u is None:
            continue
        g, items, h = u
        gw, wpos = s // GW, s % GW
        w3a[gw][:, wpos * HCOL:(wpos + 1) * HCOL] = pre["w3q"][g][h][:, :HCOL]
        w3b[gw][:, wpos * HCOL:(wpos + 1) * HCOL] = pre["w3q"][g][h][:, HCOL:]
        gp, ppos = s // GP, s % GP
        p = pin[gp][:, ppos * PIN_W:(ppos + 1) * PIN_W]
        p[0:32, PIN_F:PIN_F + EMB] = pre["F"][g]
        for i, b in enumerate(items):
            p[32 + i, PIN_F:PIN_F + EMB] = pre["tt"][b]
            p[0:32, PIN_A + i * T:PIN_A + (i + 1) * T] = actions_bf[b]
            p[32 + i, PIN_A + i * T:PIN_A + (i + 1) * T] = 1.0
    return {"w3a": w3a, "w3b": w3b, "pin": pin}


def preprocess(state, actions, timesteps, cat_ids,
               se_W1, se_b1, se_W2, se_b2,
               ae_W1, ae_b1, ae_W2, ae_b2, ae_W3, ae_b3):
    tau = _sinusoid(timesteps)
    f32 = np.float32
    pre = {"F": {}, "w3q": {}, "scale": {}, "tt": {}, "sf": {}}
    for g in sorted(set(cat_ids.tolist())):
        W2a = ae_W2[g][:EMB]
        pre["F"][g] = (ae_W1[g].astype(f32) @ W2a).astype(NPBF16)
        W3 = ae_W3[g]
        mx = float(np.abs(W3).max())
        s = 2.0 ** np.floor(np.log2(FP8MAX / mx)) if mx > 0 else 1.0
        pre["scale"][g] = s
        q = (W3 * f32(s)).astype(NPFP8)
        pre["w3q"][g] = [
            np.ascontiguousarray(
                q[:, h * OCW:(h + 1) * OCW].reshape(KCH, 128, OCW)
                .transpose(1, 0, 2).reshape(128, KCH * OCW))
            for h in range(NTHIRD)]
    for b, g in enumerate(cat_ids.tolist()):
        pre["tt"][b] = (tau[b] @ ae_W2[g][EMB:]
                        + ae_b1[g].astype(np.float64) @ ae_W2[g][:EMB]
                        + ae_b2[g]).astype(NPBF16)
        h = np.maximum(state[b, 0].astype(np.float64) @ se_W1[g] + se_b1[g], 0)
        pre["sf"][b] = (h @ se_W2[g] + se_b2[g]).astype(f32)
    return pre


def kernel(state, actions, timesteps, cat_ids,
           se_W1, se_b1, se_W2, se_b2,
           ae_W1, ae_b1, ae_W2, ae_b2, ae_W3, ae_b3):
    args = [np.asarray(a) for a in (state, actions, timesteps, cat_ids, se_W1, se_b1,
                                    se_W2, se_b2, ae_W1, ae_b1, ae_W2, ae_b2, ae_W3, ae_b3)]
    (state, actions, timesteps, cat_ids, se_W1, se_b1, se_W2, se_b2,
     ae_W1, ae_b1, ae_W2, ae_b2, ae_W3, ae_b3) = args

    pre = preprocess(*args)
    units, per_core, caps = route(cat_ids)
    actions_bf = np.ascontiguousarray(actions.transpose(0, 2, 1)).astype(NPBF16)
    in_maps = [make_inputs(per_core[c], caps, actions_bf, pre) for c in range(N_CORES)]

    nc = build(caps)
    res = run_bass_kernel_spmd(nc, in_maps, list(range(N_CORES)))

    out = np.zeros((B, T + 1, EMB), np.float32)
    for b in range(B):
        out[b, 0] = pre["sf"][b]
    for c in range(N_CORES):
        ao = res.results[c]["ao"]
        for s, u in enumerate(per_core[c]):
            if u is None:
                continue
            g, items, h = u
            cap = caps[s]
            go, opos = s // GO, s % GO
            blk = (ao[go][:, opos * OCW:opos * OCW + 4 * cap]
                   .astype(np.float32).reshape(128, 4, cap))
            inv = np.float32(1.0 / pre["scale"][g])
            for i, b in enumerate(items):
                out[b, 1:, h * OCW:(h + 1) * OCW] = (
                    blk[:, :, i * T:(i + 1) * T].transpose(2, 1, 0).reshape(T, OCW) * inv
                    + ae_b3[g][h * OCW:(h + 1) * OCW])
    return out
